# revision 2
# baseline (speedup 1.0000x reference)
"""BitLinear-STE forward via fp8 DoubleRow matmuls on 8 Trainium2 cores.

y = x @ sign(W).T with x:(4,2048,4096) f32, W:(4096,4096) f32.

Scheme: data parallel over rows (1024 rows/core).  sign(W) is exact in fp8
e4m3, so the only approximation is quantizing x.  fp8 DoubleRow matmuls run
2x the fp16 rate (measured 113.8 ns vs 226.7 ns for the same MACs), so:
  - hi-pass: e4m3(x) against all K=4096 -> rel err 2.64e-2 (over the 2e-2
    gate on its own)
  - lo-pass: e4m3(x - e4m3(x)) correction over the first C_PAIRS*256 of K.
Total err is exactly measurable host-side (inputs + rounding + accumulation
order are all deterministic): C_PAIRS=8 -> 1.8703e-2, measured bit-identical
across runs.

Schedule (per core): ob0's hi-pass runs first in DMA-arrival order (it needs
only w[ob0] 1 MiB + xh 4 MiB ~ 14 us of DMA, matching its 14.6 us of
matmuls) and is drained to SBUF; obs 1..15 run hi+lo st-outer; ob0's
lo-pass runs LAST (its xl stream has ~300 us of slack) against a pinned
w[ob0] tile, and the saved hi result is added back during the drain.
PSUM start=True zeroes the whole 2 KiB bank, so every accumulation group
gets its own bank (8 tags, reused sequentially).

Layout per core:
  xh DRAM [4096, 1024] fp8 -> SBUF [128p, 32kt, 1024rows]   (4 MiB)
  xl DRAM [C*256, 1024] fp8 -> SBUF [128p, 2C kt, 1024rows]
  wt DRAM [4096, 4096] fp8 -> per-o-blk SBUF [128p, 32kt, 256] (1 MiB)
  y  DRAM [1024, 4096] f32
"""

import numpy as np
import ml_dtypes

import concourse.mybir as mybir
import concourse.tile as tile
from concourse import bacc
from concourse.bass_utils import run_bass_kernel_spmd
from concourse.tile import add_dep_helper

N_CORES = 8
P = 128
IN_F = 4096
OUT_F = 4096
ROWS = 4 * 2048
ROWS_PER_CORE = ROWS // N_CORES      # 1024
KT = IN_F // P                       # 32 k-subtiles
QT = KT // 2                         # 16 DoubleRow k-pairs
C_PAIRS = 8                          # corrected k-pairs (of 16)
O_BLK = 256
O_BLKS = OUT_F // O_BLK              # 16
S_TILES = ROWS_PER_CORE // P         # 8

F8 = mybir.dt.float8e4
F32 = mybir.dt.float32
DR = mybir.MatmulPerfMode.DoubleRow

_NC_CACHE = {}


def _build_nc(c_pairs=C_PAIRS):
    nc = bacc.Bacc(None, target_bir_lowering=False)
    xh = nc.dram_tensor("xh", (IN_F, ROWS_PER_CORE), F8, kind="ExternalInput")
    xl = nc.dram_tensor(
        "xl", (c_pairs * 2 * P, ROWS_PER_CORE), F8, kind="ExternalInput"
    )
    wt = nc.dram_tensor("wt", (IN_F, OUT_F), F8, kind="ExternalInput")
    y = nc.dram_tensor("y", (ROWS_PER_CORE, OUT_F), F32, kind="ExternalOutput")

    xh_v = xh.rearrange("(kt p) r -> p kt r", p=P)   # [128, 32, 1024]
    xl_v = xl.rearrange("(kt p) r -> p kt r", p=P)   # [128, 2C, 1024]
    wt_v = wt.rearrange("(kt p) o -> p kt o", p=P)   # [128, 32, 4096]
    y_v = y.rearrange("(st p) o -> st p o", p=P)     # [8, 128, 4096]

    LANES = 8

    with tile.TileContext(nc) as tc:
        with (
            tc.tile_pool(name="xp", bufs=1) as xp,
            tc.tile_pool(name="wp", bufs=2) as wp,
            tc.tile_pool(name="w0p", bufs=1) as w0p,
            tc.tile_pool(name="o0p", bufs=1) as o0p,
            tc.tile_pool(name="op", bufs=4) as op,
            tc.tile_pool(name="pp", bufs=1, space="PSUM") as pp,
        ):
            lane_tails = [None] * LANES
            n_item = 0
            head_dma = None

            def chained_dma(dst, src):
                nonlocal n_item
                lane = n_item % LANES
                d = nc.scalar.dma_start(dst, src)
                dep = lane_tails[lane] if lane_tails[lane] is not None else head_dma
                if dep is not None:
                    add_dep_helper(d.ins, dep.ins, reason="load lane")
                lane_tails[lane] = d
                n_item += 1
                return d

            xh_t = xp.tile([P, KT, ROWS_PER_CORE], F8, tag="xh", name="xh")
            xl_t = xp.tile([P, 2 * c_pairs, ROWS_PER_CORE], F8, tag="xl", name="xl")
            w0_t = w0p.tile([P, KT, O_BLK], F8, tag="w0", name="w0")

            # PE warm-up while the first loads are in flight (clock ramp)
            dm = op.tile([P, 2, O_BLK], F8, tag="warm", name="warm")
            nc.any.memset(dm, 0.0)
            dps = pp.tile([P, O_BLK], F32, tag="ps0", name="warmps")
            for _ in range(12):
                nc.tensor.matmul(
                    dps, dm[:, :, :P], dm, start=True, stop=True, perf_mode=DR
                )

            # ---- DMA program: head at full bw, then 8 serial lanes in
            # first-use order: (w0,xh) pairs, w1, xl, w2.
            half = ROWS_PER_CORE // 2
            head_dma = nc.sync.dma_start(w0_t[:, 0:2, :], wt_v[:, 0:2, 0:O_BLK])
            nc.sync.dma_start(xh_t[:, 0:2, :half], xh_v[:, 0:2, :half])
            nc.sync.dma_start(xh_t[:, 0:2, half:], xh_v[:, 0:2, half:])
            nc.sync.dma_start(w0_t[:, 2:4, :], wt_v[:, 2:4, 0:O_BLK])
            nc.sync.dma_start(xh_t[:, 2:4, :], xh_v[:, 2:4, :])
            for q in range(2, QT):
                chained_dma(w0_t[:, 2 * q : 2 * q + 2, :], wt_v[:, 2 * q : 2 * q + 2, 0:O_BLK])
                chained_dma(xh_t[:, 2 * q : 2 * q + 2, :], xh_v[:, 2 * q : 2 * q + 2, :])
            w_tiles = {}
            w_tiles[1] = wp.tile([P, KT, O_BLK], F8, tag="w", name="w1")
            for q in range(QT):
                chained_dma(
                    w_tiles[1][:, 2 * q : 2 * q + 2, :],
                    wt_v[:, 2 * q : 2 * q + 2, O_BLK : 2 * O_BLK],
                )
            for q in range(c_pairs):
                chained_dma(xl_t[:, 2 * q : 2 * q + 2, :], xl_v[:, 2 * q : 2 * q + 2, :])
            w_tiles[2] = wp.tile([P, KT, O_BLK], F8, tag="w", name="w2")
            for q in range(QT):
                chained_dma(
                    w_tiles[2][:, 2 * q : 2 * q + 2, :],
                    wt_v[:, 2 * q : 2 * q + 2, 2 * O_BLK : 3 * O_BLK],
                )

            # ---- ob0 hi-pass: q-outer across 8 PSUM groups (one bank each),
            # consuming (w0[q], xh[q]) in DMA-arrival order; closed and
            # drained to SBUF so the banks free up for obs 1..15.
            o0_sb = [
                o0p.tile([P, O_BLK], F32, tag=f"o0_{st}", name=f"o0_{st}")
                for st in range(S_TILES)
            ]
            pss = [
                pp.tile([P, O_BLK], F32, tag=f"ps{st}", name=f"ps0_{st}")
                for st in range(S_TILES)
            ]
            for q in range(QT):
                for st in range(S_TILES):
                    nc.tensor.matmul(
                        pss[st],
                        xh_t[:, 2 * q : 2 * q + 2, st * P : (st + 1) * P],
                        w0_t[:, 2 * q : 2 * q + 2, :],
                        start=(q == 0),
                        stop=(q == QT - 1),
                        perf_mode=DR,
                    )
            for st in range(S_TILES):
                nc.vector.tensor_copy(o0_sb[st], pss[st])

            # ---- obs 1..15: st-outer, 16 hi + c lo per PSUM group
            for ob in range(1, O_BLKS):
                osl = slice(ob * O_BLK, (ob + 1) * O_BLK)
                if ob in w_tiles:
                    w_t = w_tiles[ob]
                else:
                    w_t = wp.tile([P, KT, O_BLK], F8, tag="w", name=f"w{ob}")
                    for q in range(QT):
                        nc.scalar.dma_start(
                            w_t[:, 2 * q : 2 * q + 2, :], wt_v[:, 2 * q : 2 * q + 2, osl]
                        )
                for st in range(S_TILES):
                    rsl = slice(st * P, (st + 1) * P)
                    ps = pp.tile([P, O_BLK], F32, tag=f"ps{st}")
                    for q in range(QT):
                        nc.tensor.matmul(
                            ps, xh_t[:, 2 * q : 2 * q + 2, rsl],
                            w_t[:, 2 * q : 2 * q + 2, :],
                            start=(q == 0), stop=False, perf_mode=DR,
                        )
                    for q in range(c_pairs):
                        nc.tensor.matmul(
                            ps, xl_t[:, 2 * q : 2 * q + 2, rsl],
                            w_t[:, 2 * q : 2 * q + 2, :],
                            start=False, stop=(q == c_pairs - 1), perf_mode=DR,
                        )
                    o_sb = op.tile([P, O_BLK], F32)
                    nc.vector.tensor_copy(o_sb, ps)
                    nc.sync.dma_start(y_v[st, :, osl], o_sb)

            # ---- deferred ob0 lo-pass: fresh PSUM groups; the saved hi
            # result is added back during the drain.  st-outer so drains
            # overlap; the last s-tile is split in half so its first half's
            # drain+DMA overlaps the second half's matmuls.
            for st in range(S_TILES):
                rsl = slice(st * P, (st + 1) * P)
                if st < S_TILES - 1:
                    ps = pp.tile([P, O_BLK], F32, tag=f"ps{st}", name=f"pslo{st}")
                    for q in range(c_pairs):
                        nc.tensor.matmul(
                            ps,
                            xl_t[:, 2 * q : 2 * q + 2, rsl],
                            w0_t[:, 2 * q : 2 * q + 2, :],
                            start=(q == 0), stop=(q == c_pairs - 1), perf_mode=DR,
                        )
                    o_sb = op.tile([P, O_BLK], F32)
                    nc.vector.tensor_tensor(o_sb, o0_sb[st], ps, mybir.AluOpType.add)
                    nc.sync.dma_start(y_v[st, :, 0:O_BLK], o_sb)
                else:
                    oh = O_BLK // 2
                    for h in range(2):
                        hsl = slice(h * oh, (h + 1) * oh)
                        ph = pp.tile(
                            [P, oh], F32, tag=f"ps{st if h else 0}", name=f"pslast{h}"
                        )
                        for q in range(c_pairs):
                            nc.tensor.matmul(
                                ph,
                                xl_t[:, 2 * q : 2 * q + 2, rsl],
                                w0_t[:, 2 * q : 2 * q + 2, hsl],
                                start=(q == 0), stop=(q == c_pairs - 1), perf_mode=DR,
                            )
                        o_sb = op.tile([P, oh], F32, tag="olast", name=f"olast{h}")
                        nc.vector.tensor_tensor(
                            o_sb, o0_sb[st][:, hsl], ph, mybir.AluOpType.add
                        )
                        nc.sync.dma_start(y_v[st, :, h * oh : (h + 1) * oh], o_sb)
    nc.finalize()
    return nc


def _get_nc():
    if "nc" not in _NC_CACHE:
        _NC_CACHE["nc"] = _build_nc()
    return _NC_CACHE["nc"]


def _prep_inputs(x, weight):
    x2 = np.ascontiguousarray(x, dtype=np.float32).reshape(ROWS, IN_F)
    xh = x2.astype(ml_dtypes.float8_e4m3)
    kc = C_PAIRS * 2 * P
    xl = (x2[:, :kc] - xh[:, :kc].astype(np.float32)).astype(ml_dtypes.float8_e4m3)
    wq = np.sign(weight.astype(np.float32)).T.astype(ml_dtypes.float8_e4m3)
    wq = np.ascontiguousarray(wq)  # [in, out]
    in_maps = []
    for c in range(N_CORES):
        rsl = slice(c * ROWS_PER_CORE, (c + 1) * ROWS_PER_CORE)
        in_maps.append(
            {
                "xh": np.ascontiguousarray(xh[rsl].T),
                "xl": np.ascontiguousarray(xl[rsl].T),
                "wt": wq,
            }
        )
    return in_maps


def _run(x, weight, trace=False, trace_cores=None):
    in_maps = _prep_inputs(x, weight)
    res = run_bass_kernel_spmd(
        _get_nc(),
        in_maps,
        core_ids=list(range(N_CORES)),
        trace=trace,
        trace_cores=trace_cores,
    )
    out = np.concatenate([res.results[c]["y"] for c in range(N_CORES)], axis=0)
    return out.reshape(4, 2048, OUT_F), res


def _run_in_subprocess(x, weight):
    """Fallback for rare transient NRT device errors: a fresh process gets a
    fresh PJRT client, which empirically recovers where in-process retries
    cannot."""
    import os
    import subprocess
    import sys
    import tempfile

    d = tempfile.mkdtemp(prefix="bitlinear_retry_")
    xp, wp, op = (os.path.join(d, f) for f in ("x.npy", "w.npy", "out.npy"))
    np.save(xp, np.ascontiguousarray(x))
    np.save(wp, np.ascontiguousarray(weight))
    code = (
        "import importlib.util, numpy as np\n"
        f"spec = importlib.util.spec_from_file_location('kernel_sub', {__file__!r})\n"
        "m = importlib.util.module_from_spec(spec)\n"
        "spec.loader.exec_module(m)\n"
        f"out, _ = m._run(np.load({xp!r}), np.load({wp!r}))\n"
        f"np.save({op!r}, out)\n"
    )
    last = None
    for _ in range(3):
        r = subprocess.run(
            [sys.executable, "-c", code], capture_output=True, timeout=900
        )
        if r.returncode == 0 and os.path.exists(op):
            return np.load(op)
        last = r
    raise RuntimeError(
        f"subprocess retries failed: {last.returncode}\n{last.stderr[-2000:].decode(errors='replace')}"
    )


def kernel(x, weight):
    try:
        out, _ = _run(x, weight, trace=False)
        return out
    except Exception:
        return _run_in_subprocess(x, weight)


# revision 3
# speedup vs baseline: 1.0304x; 1.0304x over previous
"""BitLinear-STE via fp8 DoubleRow matmuls, o-blocks processed in pairs.

Same math as kernel2 (hi-pass e4m3(x) over K=4096 + e4m3 lo correction on
the first C_PAIRS*256 of K; rel err 1.8703e-2, deterministic), but each
PSUM group covers TWO 256-col o-blocks ([128,512] = one full 2 KiB bank,
start=True zeroes the whole bank so the second half needs no start).  The
first pair runs q-outer in DMA-arrival order, needing only ~211 GB/s of
stream (w0+w1+xh over 29 us of matmuls) instead of kernel2's ~352 GB/s
knife-edge, so the startup gaps disappear and no deferred lo-pass is
needed.
"""

import numpy as np
import ml_dtypes

import concourse.mybir as mybir
import concourse.tile as tile
from concourse import bacc
from concourse.bass_utils import run_bass_kernel_spmd
from concourse.tile import add_dep_helper

N_CORES = 8
P = 128
IN_F = 4096
OUT_F = 4096
ROWS = 4 * 2048
ROWS_PER_CORE = ROWS // N_CORES      # 1024
KT = IN_F // P                       # 32 k-subtiles
QT = KT // 2                         # 16 DoubleRow k-pairs
C_PAIRS = 8                          # corrected k-pairs (of 16)
O_BLK = 256
O_BLKS = OUT_F // O_BLK              # 16
O_PAIRS = O_BLKS // 2                # 8
S_TILES = ROWS_PER_CORE // P         # 8

F8 = mybir.dt.float8e4
F32 = mybir.dt.float32
DR = mybir.MatmulPerfMode.DoubleRow

_NC_CACHE = {}


def _build_nc(c_pairs=C_PAIRS):
    nc = bacc.Bacc(None, target_bir_lowering=False)
    xh = nc.dram_tensor("xh", (IN_F, ROWS_PER_CORE), F8, kind="ExternalInput")
    xl = nc.dram_tensor(
        "xl", (c_pairs * 2 * P, ROWS_PER_CORE), F8, kind="ExternalInput"
    )
    wt = nc.dram_tensor("wt", (IN_F, OUT_F), F8, kind="ExternalInput")
    y = nc.dram_tensor("y", (ROWS_PER_CORE, OUT_F), F32, kind="ExternalOutput")

    xh_v = xh.rearrange("(kt p) r -> p kt r", p=P)   # [128, 32, 1024]
    xl_v = xl.rearrange("(kt p) r -> p kt r", p=P)   # [128, 2C, 1024]
    wt_v = wt.rearrange("(kt p) o -> p kt o", p=P)   # [128, 32, 4096]
    y_v = y.rearrange("(st p) o -> st p o", p=P)     # [8, 128, 4096]

    LANES = 8

    with tile.TileContext(nc) as tc:
        with (
            tc.tile_pool(name="xp", bufs=1) as xp,
            tc.tile_pool(name="wp", bufs=4) as wp,
            tc.tile_pool(name="op", bufs=4) as op,
            tc.tile_pool(name="pp", bufs=1, space="PSUM") as pp,
        ):
            lane_tails = [None] * LANES
            n_item = 0
            head_dma = None

            def chained_dma(dst, src):
                nonlocal n_item
                lane = n_item % LANES
                d = nc.scalar.dma_start(dst, src)
                dep = lane_tails[lane] if lane_tails[lane] is not None else head_dma
                if dep is not None:
                    add_dep_helper(d.ins, dep.ins, reason="load lane")
                lane_tails[lane] = d
                n_item += 1
                return d

            xh_t = xp.tile([P, KT, ROWS_PER_CORE], F8, tag="xh", name="xh")
            xl_t = xp.tile([P, 2 * c_pairs, ROWS_PER_CORE], F8, tag="xl", name="xl")

            # PE warm-up while the first loads are in flight (clock ramp)
            dm = op.tile([P, 2, O_BLK], F8, tag="warm", name="warm")
            nc.any.memset(dm, 0.0)
            dps = pp.tile([P, 2 * O_BLK], F32, tag="ps0", name="warmps")
            for _ in range(12):
                nc.tensor.matmul(
                    dps[:, :O_BLK], dm[:, :, :P], dm, start=True, stop=True,
                    perf_mode=DR,
                )

            def w_osl(ob):
                return slice(ob * O_BLK, (ob + 1) * O_BLK)

            # ---- DMA program.  First-use order for the q-outer pair-0 loop:
            # (w0[q], xh[q], w1[q]) per q, then xl, then w2, w3.
            w_tiles = {}
            for ob in range(4):
                w_tiles[ob] = wp.tile([P, KT, O_BLK], F8, tag="w", name=f"w{ob}")
            half = ROWS_PER_CORE // 2
            head_dma = nc.sync.dma_start(w_tiles[0][:, 0:2, :], wt_v[:, 0:2, w_osl(0)])
            nc.sync.dma_start(xh_t[:, 0:2, :half], xh_v[:, 0:2, :half])
            nc.sync.dma_start(xh_t[:, 0:2, half:], xh_v[:, 0:2, half:])
            nc.sync.dma_start(w_tiles[1][:, 0:2, :], wt_v[:, 0:2, w_osl(1)])
            for q in range(1, QT):
                chained_dma(
                    w_tiles[0][:, 2 * q : 2 * q + 2, :], wt_v[:, 2 * q : 2 * q + 2, w_osl(0)]
                )
                chained_dma(xh_t[:, 2 * q : 2 * q + 2, :], xh_v[:, 2 * q : 2 * q + 2, :])
                chained_dma(
                    w_tiles[1][:, 2 * q : 2 * q + 2, :], wt_v[:, 2 * q : 2 * q + 2, w_osl(1)]
                )
            for q in range(c_pairs):
                chained_dma(xl_t[:, 2 * q : 2 * q + 2, :], xl_v[:, 2 * q : 2 * q + 2, :])
            for ob in (2, 3):
                for q in range(QT):
                    chained_dma(
                        w_tiles[ob][:, 2 * q : 2 * q + 2, :],
                        wt_v[:, 2 * q : 2 * q + 2, w_osl(ob)],
                    )

            # ---- pair 0 (obs 0,1): q-outer across 8 [128,512] PSUM groups,
            # consuming (w0[q], xh[q], w1[q]) in DMA-arrival order.
            pbs = [
                pp.tile([P, 2 * O_BLK], F32, tag=f"ps{st}", name=f"pb0_{st}")
                for st in range(S_TILES)
            ]
            for q in range(QT):
                for h in range(2):
                    for st in range(S_TILES):
                        nc.tensor.matmul(
                            pbs[st][:, h * O_BLK : (h + 1) * O_BLK],
                            xh_t[:, 2 * q : 2 * q + 2, st * P : (st + 1) * P],
                            w_tiles[h][:, 2 * q : 2 * q + 2, :],
                            start=(q == 0 and h == 0),
                            stop=False,
                            perf_mode=DR,
                            skip_group_check=(h == 1),
                        )
            for q in range(c_pairs):
                for h in range(2):
                    for st in range(S_TILES):
                        nc.tensor.matmul(
                            pbs[st][:, h * O_BLK : (h + 1) * O_BLK],
                            xl_t[:, 2 * q : 2 * q + 2, st * P : (st + 1) * P],
                            w_tiles[h][:, 2 * q : 2 * q + 2, :],
                            start=False,
                            stop=(q == c_pairs - 1 and h == 1),
                            perf_mode=DR,
                            skip_group_check=(h == 1),
                        )
            for st in range(S_TILES):
                o_sb = op.tile([P, 2 * O_BLK], F32)
                nc.vector.tensor_copy(o_sb, pbs[st])
                nc.sync.dma_start(y_v[st, :, 0 : 2 * O_BLK], o_sb)

            # ---- pairs 1..7: st-outer, one [128,512] group per (pair, st);
            # the very last s-tile is split into two per-ob groups so its
            # first drain+DMA overlaps the second's matmuls.
            for pr in range(1, O_PAIRS):
                obs = (2 * pr, 2 * pr + 1)
                for ob in obs:
                    if ob not in w_tiles:
                        w_tiles[ob] = wp.tile([P, KT, O_BLK], F8, tag="w", name=f"w{ob}")
                        for q in range(QT):
                            nc.scalar.dma_start(
                                w_tiles[ob][:, 2 * q : 2 * q + 2, :],
                                wt_v[:, 2 * q : 2 * q + 2, w_osl(ob)],
                            )
                wa, wb = w_tiles[obs[0]], w_tiles[obs[1]]
                for st in range(S_TILES):
                    rsl = slice(st * P, (st + 1) * P)
                    last_tile = pr == O_PAIRS - 1 and st == S_TILES - 1
                    if not last_tile:
                        pb = pp.tile([P, 2 * O_BLK], F32, tag=f"ps{st}")
                        for h, w_t in enumerate((wa, wb)):
                            for q in range(QT):
                                nc.tensor.matmul(
                                    pb[:, h * O_BLK : (h + 1) * O_BLK],
                                    xh_t[:, 2 * q : 2 * q + 2, rsl],
                                    w_t[:, 2 * q : 2 * q + 2, :],
                                    start=(q == 0 and h == 0), stop=False,
                                    perf_mode=DR, skip_group_check=(h == 1),
                                )
                        for h, w_t in enumerate((wa, wb)):
                            for q in range(c_pairs):
                                nc.tensor.matmul(
                                    pb[:, h * O_BLK : (h + 1) * O_BLK],
                                    xl_t[:, 2 * q : 2 * q + 2, rsl],
                                    w_t[:, 2 * q : 2 * q + 2, :],
                                    start=False,
                                    stop=(q == c_pairs - 1 and h == 1),
                                    perf_mode=DR, skip_group_check=(h == 1),
                                )
                        o_sb = op.tile([P, 2 * O_BLK], F32)
                        nc.vector.tensor_copy(o_sb, pb)
                        nc.sync.dma_start(
                            y_v[st, :, obs[0] * O_BLK : (obs[1] + 1) * O_BLK], o_sb
                        )
                    else:
                        for h, w_t in enumerate((wa, wb)):
                            ph = pp.tile(
                                [P, O_BLK], F32, tag=f"ps{st if h else 0}",
                                name=f"pslast{h}",
                            )
                            for q in range(QT):
                                nc.tensor.matmul(
                                    ph, xh_t[:, 2 * q : 2 * q + 2, rsl],
                                    w_t[:, 2 * q : 2 * q + 2, :],
                                    start=(q == 0), stop=False, perf_mode=DR,
                                )
                            for q in range(c_pairs):
                                nc.tensor.matmul(
                                    ph, xl_t[:, 2 * q : 2 * q + 2, rsl],
                                    w_t[:, 2 * q : 2 * q + 2, :],
                                    start=False, stop=(q == c_pairs - 1),
                                    perf_mode=DR,
                                )
                            o_sb = op.tile([P, O_BLK], F32, tag="olast", name=f"olast{h}")
                            nc.vector.tensor_copy(o_sb, ph)
                            nc.sync.dma_start(y_v[st, :, w_osl(obs[h])], o_sb)
    nc.finalize()
    return nc


def _get_nc():
    if "nc" not in _NC_CACHE:
        _NC_CACHE["nc"] = _build_nc()
    return _NC_CACHE["nc"]


def _prep_inputs(x, weight):
    x2 = np.ascontiguousarray(x, dtype=np.float32).reshape(ROWS, IN_F)
    xh = x2.astype(ml_dtypes.float8_e4m3)
    kc = C_PAIRS * 2 * P
    xl = (x2[:, :kc] - xh[:, :kc].astype(np.float32)).astype(ml_dtypes.float8_e4m3)
    wq = np.sign(weight.astype(np.float32)).T.astype(ml_dtypes.float8_e4m3)
    wq = np.ascontiguousarray(wq)  # [in, out]
    in_maps = []
    for c in range(N_CORES):
        rsl = slice(c * ROWS_PER_CORE, (c + 1) * ROWS_PER_CORE)
        in_maps.append(
            {
                "xh": np.ascontiguousarray(xh[rsl].T),
                "xl": np.ascontiguousarray(xl[rsl].T),
                "wt": wq,
            }
        )
    return in_maps


def _run(x, weight, trace=False, trace_cores=None):
    in_maps = _prep_inputs(x, weight)
    res = run_bass_kernel_spmd(
        _get_nc(),
        in_maps,
        core_ids=list(range(N_CORES)),
        trace=trace,
        trace_cores=trace_cores,
    )
    out = np.concatenate([res.results[c]["y"] for c in range(N_CORES)], axis=0)
    return out.reshape(4, 2048, OUT_F), res


def _run_in_subprocess(x, weight):
    """Fallback for rare transient NRT device errors: a fresh process gets a
    fresh PJRT client, which empirically recovers where in-process retries
    cannot."""
    import os
    import subprocess
    import sys
    import tempfile

    d = tempfile.mkdtemp(prefix="bitlinear_retry_")
    xp, wp, op = (os.path.join(d, f) for f in ("x.npy", "w.npy", "out.npy"))
    np.save(xp, np.ascontiguousarray(x))
    np.save(wp, np.ascontiguousarray(weight))
    code = (
        "import importlib.util, numpy as np\n"
        f"spec = importlib.util.spec_from_file_location('kernel_sub', {__file__!r})\n"
        "m = importlib.util.module_from_spec(spec)\n"
        "spec.loader.exec_module(m)\n"
        f"out, _ = m._run(np.load({xp!r}), np.load({wp!r}))\n"
        f"np.save({op!r}, out)\n"
    )
    last = None
    for _ in range(3):
        r = subprocess.run(
            [sys.executable, "-c", code], capture_output=True, timeout=900
        )
        if r.returncode == 0 and os.path.exists(op):
            return np.load(op)
        last = r
    raise RuntimeError(
        f"subprocess retries failed: {last.returncode}\n{last.stderr[-2000:].decode(errors='replace')}"
    )


def kernel(x, weight):
    try:
        out, _ = _run(x, weight, trace=False)
        return out
    except Exception:
        return _run_in_subprocess(x, weight)
